# revision 15
# baseline (speedup 1.0000x reference)
"""Tensor-parallel TinyLlama prefill decoder on 8 Trainium2 NeuronCores.

Returns the stacked pre-RoPE KV cache [2, L, B, H, S, HD] (the only live
output of the reference's prefill forward; the final layer's attention/FFN
are dead code and are skipped).

Sharding: tensor-parallel over heads (2/core) and FFN columns (704/core);
norms replicated. The residual stream is chunked into 4 groups of 512
positions; emission interleaves attention chunk c+1/c+2 with FFN chunk c
so each chunk's AllReduce has ~2 chunks of compute to hide under, and the
PE never drains (HAM clock-gate stays warm).

Activations live transposed ([E, S]) in SBUF so every matmul contracts
along partitions without transposes; scores are computed transposed
([k, q]) so the softmax denominator falls out of the o-matmul via an
appended ones column on v. The rms-norm x^2 partition-reduction runs as
fp8e5m2 DoubleRow matmuls (2 E-tiles per pass; the mean is insensitive
to fp8 rounding), halving its PE cost.
"""

import os
from contextlib import ExitStack

import numpy as np

import concourse.bass as bass
import concourse.mybir as mybir
import concourse.tile as tile
from concourse import bacc
from concourse.bass_utils import run_bass_kernel_spmd

F16 = mybir.dt.float16
F32 = mybir.dt.float32
F8E5 = mybir.dt.float8e5
F8E4 = mybir.dt.float8e4
DR = mybir.MatmulPerfMode.DoubleRow
AF = mybir.ActivationFunctionType

# model config (hardcoded per contract)
B, S, E, H, HD, FF, L, V = 1, 2048, 2048, 16, 128, 5632, 4, 32000
ROPE_THETA = 10000.0
EPS = 1e-5
NC = 8                      # cores
HPC = H // NC               # heads per core (2)
DPC = HPC * HD              # qkv dims per core (256)
FPC = FF // NC              # ffn dims per core (704)
ET = E // 128               # E tiles (16)
EP = ET // 2                # E tile pairs (8) for fp8 DoubleRow rms
ST = S // 128               # S blocks (16)
SC = 512                    # position-chunk width
NCH = S // SC               # chunks (4)
FT = 5                      # full 128-row FF tiles; plus one 64-row tile
SCALE = float(HD) ** -0.5

_CACHE = {}


def build_kernel():
    nc = bacc.Bacc("TRN2", target_bir_lowering=False, debug=False,
                   num_devices=NC)

    # ---- DRAM I/O --------------------------------------------------------
    x0T = nc.dram_tensor("x0T", [E, S], F16, kind="ExternalInput").ap()
    wq = nc.dram_tensor("wq", [L, E, DPC], F16, kind="ExternalInput").ap()
    wk = nc.dram_tensor("wk", [L, E, DPC], F16, kind="ExternalInput").ap()
    wv = nc.dram_tensor("wv", [L, E, DPC], F16, kind="ExternalInput").ap()
    wo = nc.dram_tensor("wo", [L, DPC, E], F16, kind="ExternalInput").ap()
    wg = nc.dram_tensor("wg", [L, E, FPC], F16, kind="ExternalInput").ap()
    wu = nc.dram_tensor("wu", [L, E, FPC], F16, kind="ExternalInput").ap()
    wd = nc.dram_tensor("wd", [L, FPC, E], F16, kind="ExternalInput").ap()
    cosT = nc.dram_tensor("cosT", [HD, S], F16, kind="ExternalInput").ap()
    sinT = nc.dram_tensor("sinT", [HD, S], F16, kind="ExternalInput").ap()
    rotP = nc.dram_tensor("rotP", [HD, HD], F16, kind="ExternalInput").ap()
    triM = nc.dram_tensor("triM", [128, 128], F16, kind="ExternalInput").ap()
    idnt = nc.dram_tensor("idnt", [128, 128], F16, kind="ExternalInput").ap()
    kvo = nc.dram_tensor("kv_out", [2, L, HPC, S, HD], F16,
                         kind="ExternalOutput").ap()

    with tile.TileContext(nc) as tc, ExitStack() as ctx:
        ctx.enter_context(nc.allow_low_precision(
            reason="fp16 kernel by design; accumulation stays fp32 in PSUM"))

        # ---- persistent SBUF ---------------------------------------------
        px = ctx.enter_context(tc.tile_pool(name="px", bufs=ET))
        x_t = []
        for e in range(ET):
            t = px.tile([128, S], F16, name=f"x_{e}", tag="x")
            nc.sync.dma_start(t[:], x0T[e * 128:(e + 1) * 128, :])
            x_t.append(t)

        pc = ctx.enter_context(tc.tile_pool(name="pconst", bufs=1))
        cos_sb = pc.tile([HD, S], F16, name="cos_sb")
        sin_sb = pc.tile([HD, S], F16, name="sin_sb")
        rot_sb = pc.tile([HD, HD], F16, name="rot_sb")
        tri_sb = pc.tile([128, 128], F16, name="tri_sb")
        id_sb = pc.tile([128, 128], F16, name="id_sb")
        ones8_sb = pc.tile([128, 2, 128], F8E4, name="ones8_sb")
        ones_sb = pc.tile([128, 128], F16, name="ones_sb")
        eps_sb = pc.tile([128, 1], F32, name="eps_sb")
        nc.sync.dma_start(cos_sb[:], cosT[:])
        nc.sync.dma_start(sin_sb[:], sinT[:])
        nc.sync.dma_start(rot_sb[:], rotP[:])
        nc.sync.dma_start(tri_sb[:], triM[:])
        nc.sync.dma_start(id_sb[:], idnt[:])
        nc.gpsimd.memset(ones8_sb[:], 1.0)
        nc.gpsimd.memset(ones_sb[:], 1.0)
        nc.gpsimd.memset(eps_sb[:], EPS)

        # DRAM bounce buffers for the chunked AllReduces
        pdram = ctx.enter_context(tc.tile_pool(name="pdram", bufs=1,
                                               space="DRAM"))
        ar_in = [[pdram.tile([E, SC], F16, name=f"ar_in{ph}_{c}",
                             tag=f"ari{ph}{c}")
                  for c in range(NCH)] for ph in range(2)]
        # Shared DRAM outputs are single-writer: one tile per collective
        ar_out = {}
        for l in range(L - 1):
            for ph in range(2):
                for c in range(NCH):
                    ar_out[(l, ph, c)] = pdram.tile(
                        [E, SC], F16, name=f"ar_out{l}_{ph}_{c}",
                        addr_space="Shared", tag=f"aro{l}{ph}{c}")

        # ---- rotating work pools (SBUF) ----------------------------------
        pw = ctx.enter_context(tc.tile_pool(name="pw", bufs=2))
        pn = ctx.enter_context(tc.tile_pool(name="pn", bufs=2))
        pqk = ctx.enter_context(tc.tile_pool(name="pqk", bufs=1))
        pv = ctx.enter_context(tc.tile_pool(name="pv", bufs=8))
        pat = ctx.enter_context(tc.tile_pool(name="pat", bufs=4))
        pff = ctx.enter_context(tc.tile_pool(name="pff", bufs=3))
        pio = ctx.enter_context(tc.tile_pool(name="pio", bufs=2))

        # ---- PSUM pools (long-lived; ring-buffered by tag) ---------------
        # slots are bank-padded: 4 shared [128,512] f32 accumulators + 4
        # o-accumulator banks (doubling as transpose scratch) = 8 banks
        ppb = ctx.enter_context(tc.tile_pool(name="ppb", bufs=4,
                                             space="PSUM"))
        ppo = ctx.enter_context(tc.tile_pool(name="ppo", bufs=1,
                                             space="PSUM"))

        def mm512(nm):
            return ppb.tile([128, SC], F32, name=nm, tag="mm512", bufs=4)

        def tp128(j, nm):
            return ppo.tile([128, 128], F16, name=nm, tag=f"ops{j}",
                            bufs=1)

        def rms_chunk(l, c, tag, want_rt):
            """R [128, SC] (rows all equal rsqrt(mean(x^2)+eps)) for
            position chunk c; optionally rT [128, 4] (per-partition r
            for each 128-block of the chunk). x^2 runs in fp8e5m2 with
            DoubleRow matmuls (2 E-tiles per pass)."""
            cs = slice(c * SC, (c + 1) * SC)
            ss = mm512(f"ss_{l}_{tag}_{c}")
            if l == 0:
                # layer-0 x is tiny (x^2 below the e4m3 denormal floor):
                # plain fp16 reduction
                for e in range(ET):
                    x2 = pn.tile([128, SC], F16, name=f"x2f_{e}",
                                 tag="x2", bufs=3)
                    nc.vector.tensor_mul(x2[:], x_t[e][:, cs],
                                         x_t[e][:, cs])
                    nc.tensor.matmul(ss[:], ones_sb[:], x2[:],
                                     start=(e == 0), stop=(e == ET - 1))
            else:
                for j in range(EP):
                    x2 = pn.tile([128, 2, SC], F8E4, name=f"x2_{j}",
                                 tag="x2", bufs=3)
                    nc.vector.tensor_mul(x2[:, 0, :],
                                         x_t[2 * j][:, cs],
                                         x_t[2 * j][:, cs])
                    nc.vector.tensor_mul(x2[:, 1, :],
                                         x_t[2 * j + 1][:, cs],
                                         x_t[2 * j + 1][:, cs])
                    nc.tensor.matmul(ss[:], ones8_sb[:], x2[:],
                                     start=(j == 0), stop=(j == EP - 1),
                                     perf_mode=DR)
            R = pn.tile([128, SC], F16, name=f"R_{l}_{tag}_{c}",
                        tag=f"R{tag}", bufs=2)
            nc.scalar.activation(R[:], ss[:], AF.Abs_reciprocal_sqrt,
                                 bias=eps_sb[:], scale=1.0 / E)
            rT = None
            if want_rt:
                # per-partition r for each 128-block, via PE transpose
                rT = pn.tile([128, 4], F32, name=f"rT_{l}_{c}",
                             tag="rT", bufs=2)
                for j in range(4):
                    tp = tp128(j, f"tpr{j}")
                    nc.tensor.transpose(
                        tp[:], R[:, j * 128:(j + 1) * 128], id_sb[:])
                    nc.vector.tensor_copy(rT[:, j:j + 1], tp[:, 0:1])
            return R, rT

        def load_w_cols(dram_ap, cols, name, tag, bufs):
            """DRAM [E, cols] -> SBUF [128, ET*cols], E-tile major."""
            t = pw.tile([128, ET * cols], F16, name=name, tag=tag, bufs=bufs)
            nc.sync.dma_start(
                t[:].rearrange("p (t m) -> p t m", t=ET),
                dram_ap.rearrange("(t p) m -> p t m", p=128))
            return t

        rg = [list(range(NC))]

        # warm up the PE HAM clock-gate while x0/consts stream in
        warm = mm512("warm")
        for i in range(20):
            nc.tensor.matmul(warm[:], cos_sb[:, 0:128],
                             cos_sb[:, 0:SC], start=(i == 0), stop=(i == 19))

        # persistent v tiles [s, d | ones], one per (head, chunk) holding
        # 4 s-blocks: ones cols written once, the value region is
        # overwritten every layer (WAR tracked by tile)
        vext = [[pv.tile([128, 4, 132], F16, name=f"vx_{h}_{c}",
                         tag=f"vx{h}{c}", bufs=1)
                 for c in range(NCH)] for h in range(HPC)]
        for h in range(HPC):
            for c in range(NCH):
                nc.gpsimd.memset(vext[h][c][:, :, 128:132], 1.0)

        # per-layer state passed between the chunk closures
        state = {}

        def attn_chunk(l, c):
            act = l < L - 1
            cs = slice(c * SC, (c + 1) * SC)

            # residual add from previous layer's FFN AllReduce
            if l > 0:
                for g in range(4):
                    ld4 = pio.tile([128, 4, SC], F16, name=f"arf_{g}",
                                   tag="arl", bufs=2)
                    nc.sync.dma_start(
                        ld4[:],
                        ar_out[(l - 1, 1, c)][g * 512:(g + 1) * 512, :]
                        .rearrange("(t p) m -> p t m", p=128))
                    for i in range(4):
                        e = 4 * g + i
                        nc.vector.tensor_add(x_t[e][:, cs],
                                             x_t[e][:, cs],
                                             ld4[:, i, :])

            R1, rT1 = rms_chunk(l, c, "a", want_rt=True)
            wk_sb = state["wk"]
            wq_sb = state["wq"]
            wv_sb = state["wv"]
            wo_sb = state["wo"]
            kr_sb = state["kr"]

            # q/k projections + RoPE + k output for this chunk
            srcs = [("k", wk_sb)] + ([("q", wq_sb)] if act else [])
            for nmw, wsb in srcs:
                for h in range(HPC):
                    if act:
                        tgt = kr_sb[h] if nmw == "k" else None
                        if nmw == "q":
                            tgt = pqk.tile([128, SC], F16,
                                           name=f"qr_{l}_{h}_{c}",
                                           tag=f"qr{h}", bufs=2)
                    ps = mm512(f"qk_{nmw}_{h}_{c}")
                    for e in range(ET):
                        nc.tensor.matmul(
                            ps[:],
                            wsb[:, e * DPC + h * 128:
                                e * DPC + (h + 1) * 128],
                            x_t[e][:, cs],
                            start=(e == 0), stop=(e == ET - 1))
                    raw = pn.tile([128, SC], F16, name=f"raw_{h}",
                                  tag="qkraw", bufs=3)
                    nc.vector.tensor_mul(raw[:], ps[:], R1[:])
                    if nmw == "k":
                        # k output (pre-RoPE): [d, s] -> [s, d]
                        ko4 = pio.tile([128, 4, 128], F16,
                                       name=f"kos_{h}", tag="kosb",
                                       bufs=1)
                        for j in range(4):
                            tp = tp128(j, f"ko{j}")
                            nc.tensor.transpose(
                                tp[:], raw[:, j * 128:(j + 1) * 128],
                                id_sb[:])
                            if j % 2 == 0:
                                nc.vector.tensor_copy(ko4[:, j, :], tp[:])
                            else:
                                nc.scalar.copy(ko4[:, j, :], tp[:])
                        nc.sync.dma_start(
                            kvo[0, l, h, c * SC:(c + 1) * SC, :]
                            .rearrange("(t p) m -> p t m", p=128),
                            ko4[:])
                    if act:
                        # RoPE: t = raw*cos + (rotP.T @ raw)*sin
                        dst = kr_sb[h][:, cs] if nmw == "k" else tgt[:]
                        rp = ppo.tile([128, SC], F32,
                                      name=f"rot_{nmw}_{h}",
                                      tag="ops3", bufs=1)
                        nc.tensor.matmul(rp[:], rot_sb[:], raw[:],
                                         start=True, stop=True)
                        nc.vector.tensor_mul(dst, raw[:], cos_sb[:, cs])
                        tmp = pn.tile([128, SC], F16, name=f"rtmp_{h}",
                                      tag="rtmp", bufs=2)
                        nc.vector.tensor_mul(tmp[:], rp[:],
                                             sin_sb[:, cs])
                        nc.vector.tensor_add(dst, dst, tmp[:])
                        if nmw == "q":
                            if h == 0:
                                state["qr"] = [None] * HPC
                            state["qr"][h] = tgt
            qr_sb = state.get("qr")

            # v for this chunk's 4 blocks, [s, d] + ones col + output
            for j in range(4):
                sb = c * 4 + j
                ps = mm512(f"v_{sb}")
                for e in range(ET):
                    nc.tensor.matmul(
                        ps[:, 0:DPC], x_t[e][:, sb * 128:(sb + 1) * 128],
                        wv_sb[:, e * DPC:(e + 1) * DPC],
                        start=(e == 0), stop=(e == ET - 1))
                for h in range(HPC):
                    nc.vector.tensor_scalar_mul(
                        vext[h][c][:, j, 0:128],
                        ps[:, h * 128:(h + 1) * 128], rT1[:, j:j + 1])
            for h in range(HPC):
                nc.sync.dma_start(
                    kvo[1, l, h, c * SC:(c + 1) * SC, :]
                    .rearrange("(t p) m -> p t m", p=128),
                    vext[h][c][:, :, 0:128])

            if not act:
                return

            # ---- attention for q-chunk c (scores transposed;
            # score/exp staged 3 kb ahead of the o-matmuls) ---------------
            oT = []
            for h in range(HPC):
                ot = pqk.tile([128, SC], F16, name=f"oT_{l}_{h}_{c}",
                              tag=f"oT{h}", bufs=2)
                ops = [ppo.tile([128, 132], F32, name=f"ops{j}",
                                tag=f"ops{j}", bufs=1)
                       for j in range(4)]
                nkb = 4 * c + 4
                exq = {}

                def issue_st(kb, h=h, c=c, exq=exq):
                    # for the diagonal 512-block only q-cols >= q0 are
                    # ever read by the o-matmuls (qb >= kb)
                    j = kb - 4 * c
                    q0 = max(0, j) * 128
                    st = mm512(f"st_{h}_{kb}")
                    nc.tensor.matmul(
                        st[:, q0:], kr_sb[h][:, kb * 128:(kb + 1) * 128],
                        qr_sb[h][:, q0:], start=True, stop=True)
                    ex = pat.tile([128, SC], F16, name=f"ex_{kb}",
                                  tag="ex")
                    nc.scalar.activation(ex[:, q0:], st[:, q0:], AF.Exp,
                                         scale=SCALE)
                    if 0 <= j < 4:
                        nc.vector.tensor_mul(
                            ex[:, j * 128:(j + 1) * 128],
                            ex[:, j * 128:(j + 1) * 128], tri_sb[:])
                    exq[kb] = ex

                LOOK = 3
                for kb in range(min(LOOK, nkb)):
                    issue_st(kb)
                for kb in range(nkb):
                    if kb + LOOK < nkb:
                        issue_st(kb + LOOK)
                    ex = exq.pop(kb)
                    ckb = kb // 4
                    for j in range(4):
                        qb = 4 * c + j
                        if qb < kb:
                            continue
                        nc.tensor.matmul(
                            ops[j], ex[:, j * 128:(j + 1) * 128],
                            vext[h][ckb][:, kb % 4, :],
                            start=(kb == 0), stop=(kb == nkb - 1))
                # evacuation: batched phases to avoid engine ping-pong
                recs = []
                for j in range(4):
                    rec = pn.tile([128, 1], F32, name=f"rec{j}",
                                  tag=f"rec{j}", bufs=2)
                    nc.vector.reciprocal(rec[:], ops[j][:, 128:129])
                    recs.append(rec)
                obs = []
                for j in range(4):
                    ob = pio.tile([128, 128], F16, name=f"ob{j}",
                                  tag=f"ob{j}", bufs=2)
                    nc.vector.tensor_scalar_mul(ob[:], ops[j][:, 0:128],
                                                recs[j][:])
                    obs.append(ob)
                tps = []
                for j in range(4):
                    tp = tp128(j, f"to{j}")
                    nc.tensor.transpose(tp[:], obs[j][:], id_sb[:])
                    tps.append(tp)
                for j in range(4):
                    if j % 2 == 0:
                        nc.vector.tensor_copy(
                            ot[:, j * 128:(j + 1) * 128], tps[j][:])
                    else:
                        nc.scalar.copy(
                            ot[:, j * 128:(j + 1) * 128], tps[j][:])
                oT.append(ot)

            # ---- Wo partial for chunk c + fire AllReduce ----------------
            # psum evacuation split across ACT and DVE within each group
            for g in range(4):
                cst4 = pio.tile([128, 4, SC], F16, name=f"woc_{g}",
                                tag="cast", bufs=2)
                for i in range(4):
                    m = 4 * g + i
                    ps = mm512(f"wo_{m}")
                    for h in range(HPC):
                        nc.tensor.matmul(
                            ps[:],
                            wo_sb[:, h * E + m * 128:
                                  h * E + (m + 1) * 128],
                            oT[h][:], start=(h == 0),
                            stop=(h == HPC - 1))
                    if i % 2 == 0:
                        nc.scalar.copy(cst4[:, i, :], ps[:])
                    else:
                        nc.vector.tensor_copy(cst4[:, i, :], ps[:])
                eng = nc.scalar if g % 2 == 0 else nc.sync
                eng.dma_start(
                    ar_in[0][c][g * 512:(g + 1) * 512, :]
                    .rearrange("(t p) m -> p t m", p=128), cst4[:])
            nc.gpsimd.collective_compute(
                "AllReduce", mybir.AluOpType.add, replica_groups=rg,
                ins=[ar_in[0][c].opt()], outs=[ar_out[(l, 0, c)].opt()])

        def ffn_pre(l, c):
            '''residual add + rms + weight prefetch for chunk c; emitted
            ahead of the body so its DVE/ACT work clears the queues before
            the body's matmuls need it'''
            cs = slice(c * SC, (c + 1) * SC)
            # residual add from this layer's attention AllReduce
            for g in range(4):
                ld4 = pio.tile([128, 4, SC], F16, name=f"ara_{g}",
                               tag="arl", bufs=2)
                nc.sync.dma_start(
                    ld4[:],
                    ar_out[(l, 0, c)][g * 512:(g + 1) * 512, :]
                    .rearrange("(t p) m -> p t m", p=128))
                for i in range(4):
                    e = 4 * g + i
                    nc.vector.tensor_add(x_t[e][:, cs],
                                         x_t[e][:, cs],
                                         ld4[:, i, :])
            R2, _ = rms_chunk(l, c, "f", want_rt=False)
            return R2

        def ffn_body(l, c, R2):
            cs = slice(c * SC, (c + 1) * SC)
            wds = []
            for g in range(4):
                # [p, fm-tile, 4 m-blocks x 128] — 3D-balanceable DMA
                wds4 = pff.tile([128, FT + 1, 512], F16,
                                name=f"wds{g}", tag="wds", bufs=2)
                nc.sync.dma_start(
                    wds4[:, 0:FT, :],
                    wd[l][0:FT * 128, g * 512:(g + 1) * 512]
                    .rearrange("(t p) m -> p t m", p=128))
                nc.sync.dma_start(
                    wds4[0:FPC - FT * 128, FT, :],
                    wd[l][FT * 128:FPC, g * 512:(g + 1) * 512])
                wds.append(wds4)
            m_sb = []
            for fm in range(FT + 1):
                rows = 128 if fm < FT else FPC - FT * 128
                wgs = pff.tile([128, ET * rows], F16, name=f"wgs{fm}",
                               tag="wgs", bufs=2)
                nc.sync.dma_start(
                    wgs[:].rearrange("p (t m) -> p t m", t=ET),
                    wg[l][:, fm * 128:fm * 128 + rows].rearrange(
                        "(t p) m -> p t m", p=128))
                wus = pff.tile([128, ET * rows], F16, name=f"wus{fm}",
                               tag="wus", bufs=2)
                nc.sync.dma_start(
                    wus[:].rearrange("p (t m) -> p t m", t=ET),
                    wu[l][:, fm * 128:fm * 128 + rows].rearrange(
                        "(t p) m -> p t m", p=128))
                gp = mm512(f"g_{fm}_{c}")
                up = mm512(f"u_{fm}_{c}")
                for e in range(ET):
                    nc.tensor.matmul(
                        gp[0:rows, :],
                        wgs[:, e * rows:(e + 1) * rows],
                        x_t[e][:, cs], start=(e == 0),
                        stop=(e == ET - 1))
                for e in range(ET):
                    nc.tensor.matmul(
                        up[0:rows, :],
                        wus[:, e * rows:(e + 1) * rows],
                        x_t[e][:, cs], start=(e == 0),
                        stop=(e == ET - 1))
                gs = pff.tile([128, SC], F16, name=f"gs{fm}",
                              tag="gs", bufs=2)
                us = pff.tile([128, SC], F16, name=f"us{fm}",
                              tag="us", bufs=2)
                nc.vector.tensor_mul(gs[0:rows, :], gp[0:rows, :],
                                     R2[0:rows, :])
                nc.scalar.activation(gs[0:rows, :], gs[0:rows, :],
                                     AF.Silu)
                nc.vector.tensor_mul(us[0:rows, :], up[0:rows, :],
                                     R2[0:rows, :])
                mt = pff.tile([128, SC], F16, name=f"m_{fm}_{c}",
                              tag="mff", bufs=7)
                nc.vector.tensor_mul(mt[0:rows, :], gs[0:rows, :],
                                     us[0:rows, :])
                m_sb.append(mt)

            # down-proj partials (wd loaded 4 m-cols at a time)
            for g in range(4):
                wds4 = wds[g]
                cst4 = pio.tile([128, 4, SC], F16, name=f"dnc_{g}",
                                tag="cast", bufs=2)
                for i in range(4):
                    m = 4 * g + i
                    ps = mm512(f"dn_{m}")
                    for fm in range(FT + 1):
                        rows = 128 if fm < FT else FPC - FT * 128
                        nc.tensor.matmul(
                            ps[:],
                            wds4[0:rows, fm,
                                 i * 128:(i + 1) * 128],
                            m_sb[fm][0:rows, :],
                            start=(fm == 0), stop=(fm == FT))
                    if i % 2 == 0:
                        nc.scalar.copy(cst4[:, i, :], ps[:])
                    else:
                        nc.vector.tensor_copy(cst4[:, i, :], ps[:])
                eng = nc.scalar if g % 2 == 0 else nc.sync
                eng.dma_start(
                    ar_in[1][c][g * 512:(g + 1) * 512, :]
                    .rearrange("(t p) m -> p t m", p=128), cst4[:])
            nc.gpsimd.collective_compute(
                "AllReduce", mybir.AluOpType.add, replica_groups=rg,
                ins=[ar_in[1][c].opt()],
                outs=[ar_out[(l, 1, c)].opt()])

        for l in range(L):
            act = l < L - 1

            # per-layer weights (ring slots: wk, wq, wv [, wo])
            state["wk"] = load_w_cols(wk[l], DPC, f"wk_sb_{l}", "wsm", 4)
            state["wq"] = load_w_cols(wq[l], DPC, f"wq_sb_{l}", "wsm", 4) \
                if act else None
            state["wv"] = load_w_cols(wv[l], DPC, f"wv_sb_{l}", "wsm", 4)
            state["wo"] = None
            if act:
                wo_sb = pw.tile([128, HPC * E], F16, name=f"wo_sb_{l}",
                                tag="wsm", bufs=4)
                nc.sync.dma_start(
                    wo_sb[:].rearrange("p (t m) -> p t m", t=HPC),
                    wo[l].rearrange("(t p) m -> p t m", p=128))
                state["wo"] = wo_sb

            # per-layer k (rope'd) tiles, whole-S, written chunk by chunk
            state["kr"] = [pqk.tile([128, S], F16, name=f"kr_{l}_{h}",
                                    tag=f"kr{h}", bufs=1)
                           for h in range(HPC)] if act else [None] * HPC

            if not act:
                for c in range(NCH):
                    attn_chunk(l, c)
                continue

            # all-attention first (ARs get 3 chunks of cover), then the
            # FFN chunks run contiguously; each chunk's residual+rms (pre)
            # is emitted one slot early so its DVE/ACT work clears the
            # queues before the body's matmuls need it
            attn_chunk(l, 0)
            attn_chunk(l, 1)
            attn_chunk(l, 2)
            pre0 = ffn_pre(l, 0)
            ffn_body(l, 0, pre0)
            attn_chunk(l, 3)
            pre1 = ffn_pre(l, 1)
            pre2 = ffn_pre(l, 2)
            ffn_body(l, 1, pre1)
            pre3 = ffn_pre(l, 3)
            ffn_body(l, 2, pre2)
            ffn_body(l, 3, pre3)

    nc.compile()
    return nc


def _host_prep(inputs):
    """Fold norms into weights, build tables, TP-shard -> per-core in_maps."""
    ids = np.asarray(inputs["input_ids"]).reshape(-1)
    x0 = np.asarray(inputs["embed"])[ids]          # [S, E] fp32
    x0T = np.ascontiguousarray(x0.T).astype(np.float16)

    ln1 = np.asarray(inputs["ln1"], dtype=np.float32)   # [L, E]
    ln2 = np.asarray(inputs["ln2"], dtype=np.float32)
    wq_f = ln1[:, :, None] * np.asarray(inputs["Wq"])   # [L, E, H*HD]
    wk_f = ln1[:, :, None] * np.asarray(inputs["Wk"])
    wv_f = ln1[:, :, None] * np.asarray(inputs["Wv"])
    wg_f = ln2[:, :, None] * np.asarray(inputs["Wg"])
    wu_f = ln2[:, :, None] * np.asarray(inputs["Wu"])
    wo_f = np.asarray(inputs["Wo"])                     # [L, H*HD, E]
    wd_f = np.asarray(inputs["Wd"])                     # [L, FF, E]

    inv = 1.0 / (ROPE_THETA ** (np.arange(0, HD, 2, dtype=np.float32) / HD))
    t = np.arange(S, dtype=np.float32)
    freqs = np.outer(t, inv)                       # [S, HD/2]
    emb = np.concatenate([freqs, freqs], axis=-1)  # [S, HD]
    cosT = np.ascontiguousarray(np.cos(emb).T).astype(np.float16)
    sinT = np.ascontiguousarray(np.sin(emb).T).astype(np.float16)

    rotP = np.zeros((HD, HD), dtype=np.float16)
    half = HD // 2
    for d in range(half):
        rotP[d + half, d] = -1.0
    for d in range(half, HD):
        rotP[d - half, d] = 1.0

    triM = np.triu(np.ones((128, 128), dtype=np.float16))   # [k, q] valid
    idnt = np.eye(128, dtype=np.float16)

    in_maps = []
    for c in range(NC):
        ds = slice(c * DPC, (c + 1) * DPC)
        fs = slice(c * FPC, (c + 1) * FPC)
        in_maps.append({
            "x0T": x0T,
            "wq": np.ascontiguousarray(wq_f[:, :, ds]).astype(np.float16),
            "wk": np.ascontiguousarray(wk_f[:, :, ds]).astype(np.float16),
            "wv": np.ascontiguousarray(wv_f[:, :, ds]).astype(np.float16),
            "wo": np.ascontiguousarray(wo_f[:, ds, :]).astype(np.float16),
            "wg": np.ascontiguousarray(wg_f[:, :, fs]).astype(np.float16),
            "wu": np.ascontiguousarray(wu_f[:, :, fs]).astype(np.float16),
            "wd": np.ascontiguousarray(wd_f[:, fs, :]).astype(np.float16),
            "cosT": cosT, "sinT": sinT, "rotP": rotP,
            "triM": triM, "idnt": idnt,
        })
    return in_maps


def kernel(**inputs):
    if "nc" not in _CACHE:
        _CACHE["nc"] = build_kernel()
    nc = _CACHE["nc"]
    in_maps = _host_prep(inputs)
    trace = os.environ.get("KERNEL_TRACE") == "1"
    res = run_bass_kernel_spmd(nc, in_maps, core_ids=list(range(NC)),
                               trace=trace)
    if trace and res.exec_time_ns is not None:
        print(f"HW exec time: {res.exec_time_ns} ns")
        _CACHE["exec_time_ns"] = res.exec_time_ns
        if res.instructions_and_trace:
            print("trace:", res.instructions_and_trace[1])

    out = np.zeros((2, L, B, H, S, HD), dtype=np.float32)
    for c in range(NC):
        kv = res.results[c]["kv_out"].astype(np.float32)  # [2, L, HPC, S, HD]
        for h in range(HPC):
            out[:, :, 0, c * HPC + h] = kv[:, :, h]
    return out
